# revision 32
# baseline (speedup 1.0000x reference)
"""Bucket (block-diagonal) attention layer for Trainium2, 8 NeuronCores SPMD.

Sharding: data-parallel over batch (4) x tensor-parallel over head groups (2).
Core c = b*2 + g handles batch b, global heads [g*8, g*8+8).

Per-core math (local out dim 512 = 8 heads x 64):
  qT[dl, t] = sum_k Wq[g*512+dl, k] * x[b, t, k]  (+ bq)   [transposed layout]
  kT[dl, t] = likewise (bk dropped: constant-per-row score shifts cancel in
              softmax -- only bq enters scores via bq . k_j)
  v[t, dl]  = natural layout, with ones-columns appended per head so the
              attended matmul also produces the softmax denominator.
  scoresT[kt, qt] = matmul(lhsT=kT_head, rhs=qT_head)      (K=64)
  expT = exp(scoresT) -> bf16 (range needed: logits reach ~18, exp ~6e7;
         fp16 would overflow at 65504)
  att[qt, 0:64], den[qt] = matmul(lhsT=expT, rhs=[v_head | ones])  (bf16)
  y = att / den + (x_slice + bv)   [residual + bv folded on host]

v2 perf structure (vs naive per-quarter phases):
  - 512-token chunks; chunk N's projection matmuls are interleaved in PE
    program order with chunk N-1's attention matmuls, so the in-order PE
    engine never stalls waiting on scalar-engine exps and keeps its 2.4GHz
    p-state (any idle resets it to 1.2GHz for 3us).
  - scores for 4 heads land in one 4-bank psum tile -> ONE exp activation
    per 4 heads (fixed psum-access latency amortized 4x).
  - attended for 4 heads lands in ONE psum bank (sequential K=128
    start/stop groups at disjoint offsets; safe per zero-region rules) ->
    one strided reciprocal + one broadcast tensor_tensor multiply per 4
    heads + one residual add per bucket.
  - v psum->sbuf copies run on the otherwise-idle gpsimd engine.
  - x/xres DMAs batched into one multi-dim-AP transfer per chunk.
"""

import json
import sys

import numpy as np
import ml_dtypes

BF16 = ml_dtypes.bfloat16
FP16 = np.float16

B, S, D = 4, 4096, 1024
H, NB = 16, 32
HG = 2            # head groups (tensor parallel over heads)
NCORES = B * HG   # 8
DL = D // HG      # 512 local output dims per core
HL = H // HG      # 8 local heads
HD = D // H       # 64 head dim
BS = S // NB      # 128 bucket size
KC = D // 128     # 8 contraction chunks
NQ = 8            # token chunks processed as pipeline phases
TOKQ = S // NQ    # 512 tokens per chunk
NBI = TOKQ // BS  # 4 buckets per chunk
OD = DL // 128    # 4 out-dim partition tiles for qT/kT
VW = 68           # per-head block width in v tiles: 64 data + den-ones + pad

_built = None     # cached (nc,) so repeated kernel() calls reuse the program


def _apply_waitfix():
    """This container's walrus accepts at most ONE sem wait per instruction.
    Post-process the BIR json: hoist extra waits onto injected wait-only
    EventSemaphore instructions just before the owning instruction."""
    import concourse.bass as bass

    if getattr(bass.Bass, "_waitfix_applied", False):
        return
    orig = bass.Bass.to_json_bytes

    def _split(m):
        n = 0
        for f in m["functions"]:
            for blk in f["blocks"]:
                out = []
                for inst in blk["instructions"]:
                    si = inst.get("sync_info")
                    if si and si.get("on_wait") and len(si["on_wait"]) > 1:
                        waits = si["on_wait"]
                        si["on_wait"] = waits[-1:]
                        for k, w in enumerate(waits[:-1]):
                            out.append({
                                "debug": inst.get("debug", 0),
                                "engine": inst["engine"],
                                "ins": [],
                                "outs": [],
                                "name": f"wfix{n}_{k}_{inst['name']}",
                                "opcode": "EventSemaphore",
                                "sync_info": {"on_update": [], "on_wait": [w]},
                            })
                        n += 1
                    out.append(inst)
                blk["instructions"] = out
        return n

    def patched(self):
        m = json.loads(orig(self))
        _split(m)
        return json.dumps(m).encode()

    bass.Bass.to_json_bytes = patched
    bass.Bass._waitfix_applied = True


def _build():
    global _built
    if _built is not None:
        return _built

    _apply_waitfix()
    import concourse.bass as bass
    import concourse.tile as tile
    from concourse import mybir
    from concourse.bass import ts

    f32 = mybir.dt.float32
    f16 = mybir.dt.float16
    bf16 = mybir.dt.bfloat16
    Act = mybir.ActivationFunctionType
    Alu = mybir.AluOpType

    nc = bass.Bass()
    xt = nc.dram_tensor("xt", [D, S], f16, kind="ExternalInput")
    wq = nc.dram_tensor("wq", [D, DL], f16, kind="ExternalInput")
    wk = nc.dram_tensor("wk", [D, DL], f16, kind="ExternalInput")
    wv = nc.dram_tensor("wv", [D, DL], f16, kind="ExternalInput")
    bqt = nc.dram_tensor("bq", [128, OD], f32, kind="ExternalInput")
    xres = nc.dram_tensor("xres", [S, DL], f32, kind="ExternalInput")
    y = nc.dram_tensor("y", [S, DL], f32, kind="ExternalOutput")

    with tile.TileContext(nc) as tc:
        with (
            tc.tile_pool(name="wpool", bufs=1) as wpool,
            tc.tile_pool(name="xtp", bufs=2) as xtp,
            tc.tile_pool(name="qtp", bufs=2 * OD) as qtp,
            tc.tile_pool(name="ktp", bufs=2 * OD) as ktp,
            tc.tile_pool(name="vp", bufs=2 * NBI) as vpool,
            tc.tile_pool(name="ep", bufs=3) as epool,
            tc.tile_pool(name="rcp", bufs=3) as rcpool,
            tc.tile_pool(name="ntp", bufs=2) as ntpool,
            tc.tile_pool(name="yp", bufs=3) as ypool,
            tc.tile_pool(name="xrp", bufs=2) as xrpool,
            # PSUM: 2 (proj) + 4 (scores, one 4-bank tile) + 2 (attended)
            tc.tile_pool(name="ps_qkv", bufs=2, space="PSUM") as ps_qkv,
            tc.tile_pool(name="ps_sc", bufs=1, space="PSUM") as ps_sc,
            tc.tile_pool(name="ps_pa", bufs=2, space="PSUM") as ps_pa,
        ):
            st = {}  # chunk -> dict of live tiles

            def emit_w_dma(src, nm, eng):
                t = wpool.tile([128, KC * DL], f16, tag=nm, name=nm)
                eng.dma_start(
                    out=t[:].rearrange("p (c d) -> p c d", d=DL),
                    in_=src[:, :].rearrange("(c p) d -> p c d", p=128))
                return t

            def emit_xt_dma(it):
                t = xtp.tile([128, KC * TOKQ], f16, tag="xt", name="xta")
                tv = t[:].rearrange("p (c t) -> p c t", t=TOKQ)
                iv = xt[:, it * TOKQ:(it + 1) * TOKQ].rearrange(
                    "(c p) t -> p c t", p=128)
                nc.sync.dma_start(out=tv, in_=iv)
                st.setdefault(it, {})["xt"] = t

            def emit_xres_dma(it):
                t = xrpool.tile([128, NBI * DL], f32, tag="xr", name="xra")
                tv = t[:].rearrange("p (j d) -> p j d", d=DL)
                iv = xres[it * TOKQ:(it + 1) * TOKQ, :].rearrange(
                    "(j p) d -> p j d", p=128)
                nc.sync.dma_start(out=tv, in_=iv)
                st[it]["xr"] = t

            def proj_unit(it, od, key):
                c = st[it]
                xt_t = c["xt"]
                w_sb = wq_sb if key == "qt" else wk_sb
                p = ps_qkv.tile([128, TOKQ], f32, tag="pqkv", name="pp")
                for kk in range(KC):
                    nc.tensor.matmul(
                        p[:], w_sb[:, kk * DL + od * 128:kk * DL + (od + 1) * 128],
                        xt_t[:, ts(kk, TOKQ)],
                        start=(kk == 0), stop=(kk == KC - 1))
                t = (qtp if key == "qt" else ktp).tile(
                    [128, TOKQ], f16, tag=key, name=key)
                # psum evacuations run on DVE so the scalar engine stays a
                # pure-exp queue (exp latency gates the scores psum reuse)
                if key == "qt":
                    nc.vector.tensor_scalar_add(
                        t[:], p[:], bq_sb[:, od:od + 1])
                else:
                    nc.vector.tensor_copy(t[:], p[:])
                c.setdefault(key, {})[od] = t

            def qk_unit(it, od):
                proj_unit(it, od, "qt")
                proj_unit(it, od, "kt")

            def v_unit(it, j):
                c = st[it]
                xt_t = c["xt"]
                c.setdefault("v", {})
                p = ps_qkv.tile([128, TOKQ], f32, tag="pqkv", name="pv")
                for kk in range(KC):
                    nc.tensor.matmul(
                        p[:],
                        xt_t[:, kk * TOKQ + j * BS:kk * TOKQ + (j + 1) * BS],
                        wv_sb[:, ts(kk, DL)],
                        start=(kk == 0), stop=(kk == KC - 1))
                vt = vpool.tile([128, HL * VW], bf16, tag="v", name="vt")
                v3 = vt[:].rearrange("p (h c) -> p h c", c=VW)
                nc.gpsimd.memset(v3[:, :, HD:VW], 1.0)
                nc.vector.tensor_copy(
                    v3[:, :, 0:HD],
                    p[:].rearrange("p (h c) -> p h c", c=HD))
                c["v"][j] = vt

            def sc_half(it, j, half):
                """scores + exp for heads [half*4, half*4+4) of bucket j."""
                c = st[it]
                if half == 0:
                    c.setdefault("ex", {})[j] = epool.tile(
                        [128, HL * 128], bf16, tag="ex", name="ex")
                ex = c["ex"][j]
                sc4 = ps_sc.tile([128, 2048], f32, tag="sc", name="sc")
                col = slice(j * BS, (j + 1) * BS)
                for hi in range(4):
                    h = half * 4 + hi
                    od, po = h // 2, (h % 2) * HD
                    nc.tensor.matmul(
                        sc4[:, hi * 512:hi * 512 + 128],
                        c["kt"][od][po:po + HD, col],
                        c["qt"][od][po:po + HD, col],
                        start=True, stop=True)
                nc.scalar.activation(
                    ex[:, half * 512:(half + 1) * 512].rearrange(
                        "p (h c) -> p h c", c=128),
                    sc4[:].rearrange("p (h c) -> p h c", c=512)[:, :, 0:128],
                    Act.Exp)

            def attn_finish(it, j):
                """attended + normalize + residual + store for bucket j."""
                c = st[it]
                ex, vt = c["ex"][j], c["v"][j]
                rc = rcpool.tile([128, 2 * 4], f32, tag="rc", name="rc")
                nt = ntpool.tile([128, DL], f32, tag="nt", name="nt")
                for half in range(2):
                    pa = ps_pa.tile([128, 4 * VW], f32, tag="pa", name="pa")
                    for hi in range(4):
                        h = half * 4 + hi
                        nc.tensor.matmul(
                            pa[:, hi * VW:(hi + 1) * VW],
                            ex[:, h * 128:(h + 1) * 128],
                            vt[:, h * VW:(h + 1) * VW],
                            start=True, stop=True)
                    pa3 = pa[:].rearrange("p (h c) -> p h c", c=VW)
                    rcv = rc[:, half * 4:(half + 1) * 4].rearrange(
                        "p (h c) -> p h c", c=1)
                    nc.vector.reciprocal(rcv, pa3[:, :, HD:HD + 1])
                    outv = nt[:, half * 256:(half + 1) * 256].rearrange(
                        "p (h c) -> p h c", c=HD)
                    in0, in1 = bass.broadcast_tensor_aps(pa3[:, :, 0:HD], rcv)
                    nc.vector.tensor_tensor(outv, in0, in1, Alu.mult)
                yt = ypool.tile([128, DL], f32, tag="yt", name="yt")
                nc.gpsimd.tensor_tensor(
                    yt[:], nt[:], c["xr"][:, ts(j, DL)], Alu.add)
                tok0 = it * TOKQ
                nc.sync.dma_start(
                    out=y[tok0 + j * BS:tok0 + (j + 1) * BS, :], in_=yt[:])

            # --- software-pipelined main loop ---
            # x(0) + wq first: they gate the first projection group; wk/wv
            # only gate later groups, so their DMAs overlap compute.
            # Startup: DMA engines drain descriptors roughly in enqueue
            # order, so strictly order triggers by criticality: x(0) chunks
            # and wq quarters first (subtile deps let q-proj od0 start on
            # chunk0 + 256KB of wq), wk/wv at the back of the sync queue.
            t0 = xtp.tile([128, KC * TOKQ], f16, tag="xt", name="xta")
            t0v = t0[:].rearrange("p (c t) -> p c t", t=TOKQ)
            x0v = xt[:, 0:TOKQ].rearrange("(c p) t -> p c t", p=128)
            st[0] = {"xt": t0}
            wq_sb = wpool.tile([128, KC * DL], f16, tag="wq", name="wq")
            wqv = wq_sb[:].rearrange("p (c d) -> p c d", d=DL)
            wqi = wq[:, :].rearrange("(c p) d -> p c d", p=128)
            bq_sb = wpool.tile([128, OD], f32, tag="bq")
            nc.gpsimd.dma_start(out=t0v[:, 6:7, :], in_=x0v[:, 6:7, :])
            nc.gpsimd.dma_start(out=t0v[:, 7:8, :], in_=x0v[:, 7:8, :])
            nc.gpsimd.dma_start(out=bq_sb[:], in_=bqt[:, :])
            for od in (0, 1):
                sl = slice(od * 128, (od + 1) * 128)
                nc.scalar.dma_start(out=wqv[:, :, sl], in_=wqi[:, :, sl])
            for kk in range(4):
                nc.sync.dma_start(
                    out=t0v[:, kk:kk + 1, :], in_=x0v[:, kk:kk + 1, :])
            for kk in (4, 5):
                nc.scalar.dma_start(
                    out=t0v[:, kk:kk + 1, :], in_=x0v[:, kk:kk + 1, :])
            for od in (2, 3):
                sl = slice(od * 128, (od + 1) * 128)
                nc.scalar.dma_start(out=wqv[:, :, sl], in_=wqi[:, :, sl])
            wk_sb = emit_w_dma(wk, "wk", nc.sync)
            wv_sb = emit_w_dma(wv, "wv", nc.sync)
            pending = None  # carried attn_finish for bucket 3 of chunk it-2
            for it in range(NQ):
                if it + 1 < NQ:
                    emit_xt_dma(it + 1)
                emit_xres_dma(it)
                if pending is not None:
                    pending()
                    pending = None
                prev = it - 1
                if it == 0:
                    # match DMA arrival order (x+wq, then wk, then wv)
                    for od in range(OD):
                        proj_unit(0, od, "qt")
                    for od in range(OD):
                        proj_unit(0, od, "kt")
                    for j in range(NBI):
                        v_unit(0, j)
                    continue
                units = []
                for i in range(OD):
                    units.append(lambda od=i: qk_unit(it, od))
                    units.append(lambda j=i: v_unit(it, j))
                after = {}
                if prev >= 0:
                    after = {
                        0: [lambda: sc_half(prev, 0, 0)],
                        1: [lambda: sc_half(prev, 0, 1)],
                        2: [lambda: sc_half(prev, 1, 0),
                            lambda: attn_finish(prev, 0)],
                        3: [lambda: sc_half(prev, 1, 1)],
                        4: [lambda: sc_half(prev, 2, 0),
                            lambda: attn_finish(prev, 1)],
                        5: [lambda: sc_half(prev, 2, 1)],
                        6: [lambda: sc_half(prev, 3, 0),
                            lambda: attn_finish(prev, 2)],
                        7: [lambda: sc_half(prev, 3, 1)],
                    }
                    pending = lambda p=prev: attn_finish(p, 3)
                for ui, u in enumerate(units):
                    u()
                    for part in after.get(ui, ()):
                        part()
            # drain: bucket 3 of chunk NQ-2, then all of chunk NQ-1
            if pending is not None:
                pending()
            last = NQ - 1
            for j in range(NBI):
                sc_half(last, j, 0)
                sc_half(last, j, 1)
                attn_finish(last, j)

    _built = nc
    return nc


def _prep_in_maps(x, Wq, bq, Wk, bk, Wv, bv):
    x = np.asarray(x, np.float32)
    Wq = np.asarray(Wq, np.float32)
    Wv = np.asarray(Wv, np.float32)
    Wk = np.asarray(Wk, np.float32)
    bq = np.asarray(bq, np.float32)
    bv = np.asarray(bv, np.float32)

    xt_b = [np.ascontiguousarray(x[b].T).astype(FP16) for b in range(B)]
    wq_g, wk_g, wv_g, bq_g = [], [], [], []
    for g in range(HG):
        sl = slice(g * DL, (g + 1) * DL)
        wq_g.append(np.ascontiguousarray(Wq[sl, :].T).astype(FP16))
        wk_g.append(np.ascontiguousarray(Wk[sl, :].T).astype(FP16))
        wv_g.append(np.ascontiguousarray(Wv[sl, :].T).astype(FP16))
        bq_g.append(np.ascontiguousarray(
            bq[sl].reshape(DL // 128, 128).T).astype(np.float32))

    in_maps = []
    for c in range(NCORES):
        b, g = c // HG, c % HG
        sl = slice(g * DL, (g + 1) * DL)
        xres = (x[b][:, sl] + bv[None, sl]).astype(np.float32)
        in_maps.append({
            "xt": xt_b[b], "wq": wq_g[g], "wk": wk_g[g], "wv": wv_g[g],
            "bq": bq_g[g], "xres": np.ascontiguousarray(xres),
        })
    return in_maps


def _gather(results):
    out = np.empty((B, S, D), np.float32)
    for c, r in enumerate(results):
        b, g = c // HG, c % HG
        out[b, :, g * DL:(g + 1) * DL] = r["y"]
    return out


def _run(inputs, trace=False, trace_cores=None):
    nc = _build()
    from concourse.bass_utils import run_bass_kernel_spmd

    in_maps = _prep_in_maps(**inputs)
    res = run_bass_kernel_spmd(
        nc, in_maps, core_ids=list(range(NCORES)), trace=trace,
        trace_cores=trace_cores)
    return _gather(res.results), res


def kernel(**inputs):
    out, _ = _run(inputs, trace=False)
    return out


def kernel_traced(trace_cores=None, **inputs):
    """For test.py: returns (output, BassKernelResults with exec_time_ns)."""
    import types
    import trn_agent_boot.trn_boot as tb

    if "antenv.axon_hooks" not in sys.modules:
        hooks = types.ModuleType("antenv.axon_hooks")
        state = [None]
        hooks.set_axon_ntff_profile_hook = lambda h: state.__setitem__(0, h)
        hooks.get_axon_ntff_profile_hook = lambda: state[0]
        sys.modules["antenv.axon_hooks"] = hooks
        hooks.set_axon_ntff_profile_hook(
            tb._ntff_profile_via_ctypes("/opt/axon/libaxon_pjrt.so"))
    return _run(inputs, trace=True, trace_cores=trace_cores)


# revision 33
# speedup vs baseline: 1.0117x; 1.0117x over previous
"""Bucket (block-diagonal) attention layer for Trainium2, 8 NeuronCores SPMD.

Sharding: data-parallel over batch (4) x tensor-parallel over head groups (2).
Core c = b*2 + g handles batch b, global heads [g*8, g*8+8).

Per-core math (local out dim 512 = 8 heads x 64):
  qT[dl, t] = sum_k Wq[g*512+dl, k] * x[b, t, k]  (+ bq)   [transposed layout]
  kT[dl, t] = likewise (bk dropped: constant-per-row score shifts cancel in
              softmax -- only bq enters scores via bq . k_j)
  v[t, dl]  = natural layout, with ones-columns appended per head so the
              attended matmul also produces the softmax denominator.
  scoresT[kt, qt] = matmul(lhsT=kT_head, rhs=qT_head)      (K=64)
  expT = exp(scoresT) -> bf16 (range needed: logits reach ~18, exp ~6e7;
         fp16 would overflow at 65504)
  att[qt, 0:64], den[qt] = matmul(lhsT=expT, rhs=[v_head | ones])  (bf16)
  y = att / den + (x_slice + bv)   [residual + bv folded on host]

v2 perf structure (vs naive per-quarter phases):
  - 512-token chunks; chunk N's projection matmuls are interleaved in PE
    program order with chunk N-1's attention matmuls, so the in-order PE
    engine never stalls waiting on scalar-engine exps and keeps its 2.4GHz
    p-state (any idle resets it to 1.2GHz for 3us).
  - scores for 4 heads land in one 4-bank psum tile -> ONE exp activation
    per 4 heads (fixed psum-access latency amortized 4x).
  - attended for 4 heads lands in ONE psum bank (sequential K=128
    start/stop groups at disjoint offsets; safe per zero-region rules) ->
    one strided reciprocal + one broadcast tensor_tensor multiply per 4
    heads + one residual add per bucket.
  - v psum->sbuf copies run on the otherwise-idle gpsimd engine.
  - x/xres DMAs batched into one multi-dim-AP transfer per chunk.
"""

import json
import sys

import numpy as np
import ml_dtypes

BF16 = ml_dtypes.bfloat16
FP16 = np.float16

B, S, D = 4, 4096, 1024
H, NB = 16, 32
HG = 2            # head groups (tensor parallel over heads)
NCORES = B * HG   # 8
DL = D // HG      # 512 local output dims per core
HL = H // HG      # 8 local heads
HD = D // H       # 64 head dim
BS = S // NB      # 128 bucket size
KC = D // 128     # 8 contraction chunks
NQ = 8            # token chunks processed as pipeline phases
TOKQ = S // NQ    # 512 tokens per chunk
NBI = TOKQ // BS  # 4 buckets per chunk
OD = DL // 128    # 4 out-dim partition tiles for qT/kT
VW = 68           # per-head block width in v tiles: 64 data + den-ones + pad

_built = None     # cached (nc,) so repeated kernel() calls reuse the program


def _apply_waitfix():
    """This container's walrus accepts at most ONE sem wait per instruction.
    Post-process the BIR json: hoist extra waits onto injected wait-only
    EventSemaphore instructions just before the owning instruction."""
    import concourse.bass as bass

    if getattr(bass.Bass, "_waitfix_applied", False):
        return
    orig = bass.Bass.to_json_bytes

    def _split(m):
        n = 0
        for f in m["functions"]:
            for blk in f["blocks"]:
                out = []
                for inst in blk["instructions"]:
                    si = inst.get("sync_info")
                    if si and si.get("on_wait") and len(si["on_wait"]) > 1:
                        waits = si["on_wait"]
                        si["on_wait"] = waits[-1:]
                        for k, w in enumerate(waits[:-1]):
                            out.append({
                                "debug": inst.get("debug", 0),
                                "engine": inst["engine"],
                                "ins": [],
                                "outs": [],
                                "name": f"wfix{n}_{k}_{inst['name']}",
                                "opcode": "EventSemaphore",
                                "sync_info": {"on_update": [], "on_wait": [w]},
                            })
                        n += 1
                    out.append(inst)
                blk["instructions"] = out
        return n

    def patched(self):
        m = json.loads(orig(self))
        _split(m)
        return json.dumps(m).encode()

    bass.Bass.to_json_bytes = patched
    bass.Bass._waitfix_applied = True


def _build():
    global _built
    if _built is not None:
        return _built

    _apply_waitfix()
    import concourse.bass as bass
    import concourse.tile as tile
    from concourse import mybir
    from concourse.bass import ts

    f32 = mybir.dt.float32
    f16 = mybir.dt.float16
    bf16 = mybir.dt.bfloat16
    Act = mybir.ActivationFunctionType
    Alu = mybir.AluOpType

    nc = bass.Bass()
    xt = nc.dram_tensor("xt", [D, S], f16, kind="ExternalInput")
    wq = nc.dram_tensor("wq", [D, DL], f16, kind="ExternalInput")
    wk = nc.dram_tensor("wk", [D, DL], f16, kind="ExternalInput")
    wv = nc.dram_tensor("wv", [D, DL], f16, kind="ExternalInput")
    bqt = nc.dram_tensor("bq", [128, OD], f32, kind="ExternalInput")
    xres = nc.dram_tensor("xres", [S, DL], f32, kind="ExternalInput")
    y = nc.dram_tensor("y", [S, DL], f32, kind="ExternalOutput")

    with tile.TileContext(nc) as tc:
        with (
            tc.tile_pool(name="wpool", bufs=1) as wpool,
            tc.tile_pool(name="xtp", bufs=2) as xtp,
            tc.tile_pool(name="qtp", bufs=2 * OD) as qtp,
            tc.tile_pool(name="ktp", bufs=2 * OD) as ktp,
            tc.tile_pool(name="vp", bufs=2 * NBI) as vpool,
            tc.tile_pool(name="ep", bufs=3) as epool,
            tc.tile_pool(name="rcp", bufs=3) as rcpool,
            tc.tile_pool(name="ntp", bufs=2) as ntpool,
            tc.tile_pool(name="yp", bufs=3) as ypool,
            tc.tile_pool(name="xrp", bufs=2) as xrpool,
            # PSUM: 2 (proj) + 4 (scores, one 4-bank tile) + 2 (attended)
            tc.tile_pool(name="ps_qkv", bufs=2, space="PSUM") as ps_qkv,
            tc.tile_pool(name="ps_sc", bufs=1, space="PSUM") as ps_sc,
            tc.tile_pool(name="ps_pa", bufs=2, space="PSUM") as ps_pa,
        ):
            st = {}  # chunk -> dict of live tiles

            def emit_w_dma(src, nm, eng):
                t = wpool.tile([128, KC * DL], f16, tag=nm, name=nm)
                eng.dma_start(
                    out=t[:].rearrange("p (c d) -> p c d", d=DL),
                    in_=src[:, :].rearrange("(c p) d -> p c d", p=128))
                return t

            def emit_xt_dma(it):
                t = xtp.tile([128, KC * TOKQ], f16, tag="xt", name="xta")
                tv = t[:].rearrange("p (c t) -> p c t", t=TOKQ)
                iv = xt[:, it * TOKQ:(it + 1) * TOKQ].rearrange(
                    "(c p) t -> p c t", p=128)
                nc.sync.dma_start(out=tv, in_=iv)
                st.setdefault(it, {})["xt"] = t

            def emit_xres_dma(it):
                t = xrpool.tile([128, NBI * DL], f32, tag="xr", name="xra")
                tv = t[:].rearrange("p (j d) -> p j d", d=DL)
                iv = xres[it * TOKQ:(it + 1) * TOKQ, :].rearrange(
                    "(j p) d -> p j d", p=128)
                nc.sync.dma_start(out=tv, in_=iv)
                st[it]["xr"] = t

            def proj_unit(it, od, key):
                c = st[it]
                xt_t = c["xt"]
                w_sb = wq_sb if key == "qt" else wk_sb
                p = ps_qkv.tile([128, TOKQ], f32, tag="pqkv", name="pp")
                for kk in range(KC):
                    nc.tensor.matmul(
                        p[:], w_sb[:, kk * DL + od * 128:kk * DL + (od + 1) * 128],
                        xt_t[:, ts(kk, TOKQ)],
                        start=(kk == 0), stop=(kk == KC - 1))
                t = (qtp if key == "qt" else ktp).tile(
                    [128, TOKQ], f16, tag=key, name=key)
                # psum evacuations run on DVE so the scalar engine stays a
                # pure-exp queue (exp latency gates the scores psum reuse)
                if key == "qt":
                    nc.vector.tensor_scalar_add(
                        t[:], p[:], bq_sb[:, od:od + 1])
                else:
                    nc.vector.tensor_copy(t[:], p[:])
                c.setdefault(key, {})[od] = t

            def qk_unit(it, od):
                proj_unit(it, od, "qt")
                proj_unit(it, od, "kt")

            def v_unit(it, j):
                c = st[it]
                xt_t = c["xt"]
                c.setdefault("v", {})
                p = ps_qkv.tile([128, TOKQ], f32, tag="pqkv", name="pv")
                for kk in range(KC):
                    nc.tensor.matmul(
                        p[:],
                        xt_t[:, kk * TOKQ + j * BS:kk * TOKQ + (j + 1) * BS],
                        wv_sb[:, ts(kk, DL)],
                        start=(kk == 0), stop=(kk == KC - 1))
                vt = vpool.tile([128, HL * VW], bf16, tag="v", name="vt")
                v3 = vt[:].rearrange("p (h c) -> p h c", c=VW)
                nc.gpsimd.memset(v3[:, :, HD:VW], 1.0)
                nc.vector.tensor_copy(
                    v3[:, :, 0:HD],
                    p[:].rearrange("p (h c) -> p h c", c=HD))
                c["v"][j] = vt

            def sc_half(it, j, half):
                """scores + exp for heads [half*4, half*4+4) of bucket j."""
                c = st[it]
                if half == 0:
                    c.setdefault("ex", {})[j] = epool.tile(
                        [128, HL * 128], bf16, tag="ex", name="ex")
                ex = c["ex"][j]
                sc4 = ps_sc.tile([128, 2048], f32, tag="sc", name="sc")
                col = slice(j * BS, (j + 1) * BS)
                for hi in range(4):
                    h = half * 4 + hi
                    od, po = h // 2, (h % 2) * HD
                    nc.tensor.matmul(
                        sc4[:, hi * 512:hi * 512 + 128],
                        c["kt"][od][po:po + HD, col],
                        c["qt"][od][po:po + HD, col],
                        start=True, stop=True)
                nc.scalar.activation(
                    ex[:, half * 512:(half + 1) * 512].rearrange(
                        "p (h c) -> p h c", c=128),
                    sc4[:].rearrange("p (h c) -> p h c", c=512)[:, :, 0:128],
                    Act.Exp)

            def attn_finish(it, j):
                """attended + normalize + residual + store for bucket j."""
                c = st[it]
                ex, vt = c["ex"][j], c["v"][j]
                rc = rcpool.tile([128, 2 * 4], f32, tag="rc", name="rc")
                nt = ntpool.tile([128, DL], f32, tag="nt", name="nt")
                for half in range(2):
                    pa = ps_pa.tile([128, 4 * VW], f32, tag="pa", name="pa")
                    for hi in range(4):
                        h = half * 4 + hi
                        nc.tensor.matmul(
                            pa[:, hi * VW:(hi + 1) * VW],
                            ex[:, h * 128:(h + 1) * 128],
                            vt[:, h * VW:(h + 1) * VW],
                            start=True, stop=True)
                    pa3 = pa[:].rearrange("p (h c) -> p h c", c=VW)
                    rcv = rc[:, half * 4:(half + 1) * 4].rearrange(
                        "p (h c) -> p h c", c=1)
                    nc.vector.reciprocal(rcv, pa3[:, :, HD:HD + 1])
                    outv = nt[:, half * 256:(half + 1) * 256].rearrange(
                        "p (h c) -> p h c", c=HD)
                    in0, in1 = bass.broadcast_tensor_aps(pa3[:, :, 0:HD], rcv)
                    nc.vector.tensor_tensor(outv, in0, in1, Alu.mult)
                yt = ypool.tile([128, DL], f32, tag="yt", name="yt")
                nc.gpsimd.tensor_tensor(
                    yt[:], nt[:], c["xr"][:, ts(j, DL)], Alu.add)
                tok0 = it * TOKQ
                nc.sync.dma_start(
                    out=y[tok0 + j * BS:tok0 + (j + 1) * BS, :], in_=yt[:])

            # --- software-pipelined main loop ---
            # x(0) + wq first: they gate the first projection group; wk/wv
            # only gate later groups, so their DMAs overlap compute.
            # Startup: DMA engines drain descriptors roughly in enqueue
            # order, so strictly order triggers by criticality: x(0) chunks
            # and wq quarters first (subtile deps let q-proj od0 start on
            # chunk0 + 256KB of wq), wk/wv at the back of the sync queue.
            t0 = xtp.tile([128, KC * TOKQ], f16, tag="xt", name="xta")
            t0v = t0[:].rearrange("p (c t) -> p c t", t=TOKQ)
            x0v = xt[:, 0:TOKQ].rearrange("(c p) t -> p c t", p=128)
            st[0] = {"xt": t0}
            wq_sb = wpool.tile([128, KC * DL], f16, tag="wq", name="wq")
            wqv = wq_sb[:].rearrange("p (c d) -> p c d", d=DL)
            wqi = wq[:, :].rearrange("(c p) d -> p c d", p=128)
            bq_sb = wpool.tile([128, OD], f32, tag="bq")
            nc.sync.dma_start(out=t0v, in_=x0v)
            nc.scalar.dma_start(out=wqv, in_=wqi)
            nc.scalar.dma_start(out=bq_sb[:], in_=bqt[:, :])
            wk_sb = emit_w_dma(wk, "wk", nc.sync)
            wv_sb = emit_w_dma(wv, "wv", nc.sync)
            pending = None  # carried attn_finish for bucket 3 of chunk it-2
            for it in range(NQ):
                if it + 1 < NQ:
                    emit_xt_dma(it + 1)
                emit_xres_dma(it)
                if pending is not None:
                    pending()
                    pending = None
                prev = it - 1
                if it == 0:
                    # match DMA arrival order (x+wq, then wk, then wv)
                    for od in range(OD):
                        proj_unit(0, od, "qt")
                    for od in range(OD):
                        proj_unit(0, od, "kt")
                    for j in range(NBI):
                        v_unit(0, j)
                    continue
                units = []
                for i in range(OD):
                    units.append(lambda od=i: qk_unit(it, od))
                    units.append(lambda j=i: v_unit(it, j))
                after = {}
                if prev >= 0:
                    after = {
                        0: [lambda: sc_half(prev, 0, 0)],
                        1: [lambda: sc_half(prev, 0, 1)],
                        2: [lambda: sc_half(prev, 1, 0),
                            lambda: attn_finish(prev, 0)],
                        3: [lambda: sc_half(prev, 1, 1)],
                        4: [lambda: sc_half(prev, 2, 0),
                            lambda: attn_finish(prev, 1)],
                        5: [lambda: sc_half(prev, 2, 1)],
                        6: [lambda: sc_half(prev, 3, 0),
                            lambda: attn_finish(prev, 2)],
                        7: [lambda: sc_half(prev, 3, 1)],
                    }
                    pending = lambda p=prev: attn_finish(p, 3)
                for ui, u in enumerate(units):
                    u()
                    for part in after.get(ui, ()):
                        part()
            # drain: bucket 3 of chunk NQ-2, then all of chunk NQ-1
            if pending is not None:
                pending()
            last = NQ - 1
            for j in range(NBI):
                sc_half(last, j, 0)
                sc_half(last, j, 1)
                attn_finish(last, j)

    _built = nc
    return nc


def _prep_in_maps(x, Wq, bq, Wk, bk, Wv, bv):
    x = np.asarray(x, np.float32)
    Wq = np.asarray(Wq, np.float32)
    Wv = np.asarray(Wv, np.float32)
    Wk = np.asarray(Wk, np.float32)
    bq = np.asarray(bq, np.float32)
    bv = np.asarray(bv, np.float32)

    xt_b = [np.ascontiguousarray(x[b].T).astype(FP16) for b in range(B)]
    wq_g, wk_g, wv_g, bq_g = [], [], [], []
    for g in range(HG):
        sl = slice(g * DL, (g + 1) * DL)
        wq_g.append(np.ascontiguousarray(Wq[sl, :].T).astype(FP16))
        wk_g.append(np.ascontiguousarray(Wk[sl, :].T).astype(FP16))
        wv_g.append(np.ascontiguousarray(Wv[sl, :].T).astype(FP16))
        bq_g.append(np.ascontiguousarray(
            bq[sl].reshape(DL // 128, 128).T).astype(np.float32))

    in_maps = []
    for c in range(NCORES):
        b, g = c // HG, c % HG
        sl = slice(g * DL, (g + 1) * DL)
        xres = (x[b][:, sl] + bv[None, sl]).astype(np.float32)
        in_maps.append({
            "xt": xt_b[b], "wq": wq_g[g], "wk": wk_g[g], "wv": wv_g[g],
            "bq": bq_g[g], "xres": np.ascontiguousarray(xres),
        })
    return in_maps


def _gather(results):
    out = np.empty((B, S, D), np.float32)
    for c, r in enumerate(results):
        b, g = c // HG, c % HG
        out[b, :, g * DL:(g + 1) * DL] = r["y"]
    return out


def _run(inputs, trace=False, trace_cores=None):
    nc = _build()
    from concourse.bass_utils import run_bass_kernel_spmd

    in_maps = _prep_in_maps(**inputs)
    res = run_bass_kernel_spmd(
        nc, in_maps, core_ids=list(range(NCORES)), trace=trace,
        trace_cores=trace_cores)
    return _gather(res.results), res


def kernel(**inputs):
    out, _ = _run(inputs, trace=False)
    return out


def kernel_traced(trace_cores=None, **inputs):
    """For test.py: returns (output, BassKernelResults with exec_time_ns)."""
    import types
    import trn_agent_boot.trn_boot as tb

    if "antenv.axon_hooks" not in sys.modules:
        hooks = types.ModuleType("antenv.axon_hooks")
        state = [None]
        hooks.set_axon_ntff_profile_hook = lambda h: state.__setitem__(0, h)
        hooks.get_axon_ntff_profile_hook = lambda: state[0]
        sys.modules["antenv.axon_hooks"] = hooks
        hooks.set_axon_ntff_profile_hook(
            tb._ntff_profile_via_ctypes("/opt/axon/libaxon_pjrt.so"))
    return _run(inputs, trace=True, trace_cores=trace_cores)
